# revision 1
# baseline (speedup 1.0000x reference)
"""Trainium2 Bass kernel for the dense_cnn problem.

Math (per sample, C=256, H=W=56, HW=3136, G=2, K=128):
  t1 = p1*x
  t2 = t1[c,hw] @ p2[hw,k]                  (computed transposed: t2T[k,c])
  t3 = t1 @ x.T / sqrt(hw)                  (computed transposed: t3T[d,c])
  t4 = grouped dilated 3x1 conv of t2 (only middle kw tap contributes)
  t5 = w5 @ x
  t8 = grouped dilated 3x3 conv of x (9 shifted block-diag matmuls)
  t9 = max(t5, t8)
  out = (t4 - t3/sqrt(hw)) @ t9 / sqrt(c)

Distribution: pure data-parallel over batch, 4 samples per core x 8 cores.
Layouts: hw-contraction ops run on a transposed copy of x ([hw, c], padded to
3200 rows); spatial ops run on a width-padded copy ([c, 56 x 62]) so all conv
taps become column-shifted matmuls with correct zero boundaries.
All large matmuls use float32r (full PE rate at free-dim >= 256).
"""

import numpy as np

import concourse.bass as bass
import concourse.tile as tile
from concourse import bacc, mybir
from concourse.bass_utils import run_bass_kernel_spmd

N, C, H, W = 32, 256, 56, 56
HW = H * W              # 3136
PW = W + 6              # width padded by 3 each side: 62
HWP = H * PW            # 3472
NCORE = 8
SPC = N // NCORE        # samples per core: 4
NCHUNK = 25             # hw-contraction chunks of 128 (rows padded to 3200)
HWPAD = NCHUNK * 128    # 3200
NCH2 = 7                # phase-2 column chunks
CH_SP = HWP // NCH2     # 496 padded cols per chunk (8 image rows)
CH_OUT = CH_SP - 48     # 448 compact cols per chunk
ROWS_PER_CHUNK = 8
XM = 192                # zero margin around each padded half (> max |shift| 189)
HWPM = HWP + 2 * XM     # 3856

F32 = mybir.dt.float32
F32R = mybir.dt.float32r
MUL = mybir.AluOpType.mult
ADD = mybir.AluOpType.add

_PROGRAM_CACHE: dict = {}


def _r(ap):
    return ap.bitcast(F32R)


def _build_program():
    nc = bacc.Bacc("TRN2", target_bir_lowering=False, debug=False,
                   num_devices=NCORE)

    d = {}
    def din(name, shape, dt=F32):
        d[name] = nc.dram_tensor(name, list(shape), dt, kind="ExternalInput").ap()
    din("xpad", (SPC, 2, 128, HWPM), F32R)
    din("xt", (SPC, NCHUNK, 128, 256), F32R)
    din("p1t", (NCHUNK, 128, 256))
    din("p2f", (NCHUNK, 128, 128), F32R)
    din("w4t", (3, 2, 128, 128))
    din("w8t", (3, 3, 2, 128, 128), F32R)
    din("w5t", (2, 2, 128, 128), F32R)
    din("ident", (128, 128))
    out_dram = nc.dram_tensor("out", [SPC, 2, 128, HW], F32,
                              kind="ExternalOutput").ap()

    with tile.TileContext(nc) as tc:
        _emit(tc, nc, d, out_dram)
    nc.compile()
    return nc


def _emit(tc, nc, d, out_dram):
    from contextlib import ExitStack
    ctx = ExitStack()
    with ctx:
        const = ctx.enter_context(tc.tile_pool(name="const", bufs=1))
        xt_pool = ctx.enter_context(tc.tile_pool(name="xt", bufs=4))
        t1_pool = ctx.enter_context(tc.tile_pool(name="t1", bufs=4))
        xpad_pool = ctx.enter_context(tc.tile_pool(name="xpad", bufs=2))
        t9_pool = ctx.enter_context(tc.tile_pool(name="t9", bufs=4))
        sb_small = ctx.enter_context(tc.tile_pool(name="sbs", bufs=2))
        out_pool = ctx.enter_context(tc.tile_pool(name="outp", bufs=3))
        # PSUM budget (8 banks total): acc 3 + smps(shared with out) 2 +
        # t8 2 + t5 1 = 8
        acc_ps = ctx.enter_context(tc.tile_pool(name="accps", bufs=1, space="PSUM"))
        small_ps = ctx.enter_context(tc.tile_pool(name="smps", bufs=2, space="PSUM"))
        sp_ps = ctx.enter_context(tc.tile_pool(name="spps", bufs=1, space="PSUM"))

        # ---- load constants -------------------------------------------------
        p1t_sb = const.tile([128, NCHUNK * 256], F32)
        nc.sync.dma_start(
            p1t_sb[:].rearrange("p (i f) -> p i f", i=NCHUNK),
            d["p1t"].rearrange("i p f -> p i f"))
        p2f_sb = const.tile([128, NCHUNK * 128], F32R)
        nc.sync.dma_start(
            p2f_sb[:].rearrange("p (i f) -> p i f", i=NCHUNK),
            d["p2f"].rearrange("i p f -> p i f"))
        w4t_sb = const.tile([128, 6 * 128], F32)
        nc.sync.dma_start(
            w4t_sb[:].rearrange("p (i f) -> p i f", i=6),
            d["w4t"].rearrange("a b p f -> p (a b) f"))
        w8t_sb = const.tile([128, 18 * 128], F32R)
        nc.sync.dma_start(
            w8t_sb[:].rearrange("p (i f) -> p i f", i=18),
            d["w8t"].rearrange("a b c p f -> p (a b c) f"))
        w5t_sb = const.tile([128, 4 * 128], F32R)
        nc.sync.dma_start(
            w5t_sb[:].rearrange("p (i f) -> p i f", i=4),
            d["w5t"].rearrange("a b p f -> p (a b) f"))
        id_sb = const.tile([128, 128], F32)
        nc.sync.dma_start(id_sb[:], d["ident"])
        # padded t2 staging ([128, 2 x 134], pad cols stay zero)
        t2p_sb = const.tile([128, 2 * 134], F32)
        nc.gpsimd.memset(t2p_sb[:], 0.0)

        inv56 = float(1.0 / np.float32(np.sqrt(np.float32(HW))))
        inv16 = float(1.0 / np.float32(np.sqrt(np.float32(C))))

        # tap order for t8: (1,1) first (always full coverage -> start=True)
        taps = [(1, 1)] + [(kh, kw) for kh in range(3) for kw in range(3)
                           if (kh, kw) != (1, 1)]

        for s in range(SPC):
            # ---- phase 1: hw-contraction accumulations ----------------------
            t2T_ps = acc_ps.tile([128, 256], F32, tag="t2T")
            t3T_ps = [acc_ps.tile([128, 256], F32, name=f"t3T{g}", tag=f"t3T{g}")
                      for g in range(2)]
            for i in range(NCHUNK):
                xt_t = xt_pool.tile([128, 256], F32R)
                nc.sync.dma_start(xt_t[:], d["xt"][s, i])
                t1_t = t1_pool.tile([128, 256], F32R)
                nc.vector.tensor_mul(t1_t[:], xt_t[:],
                                     p1t_sb[:, i * 256:(i + 1) * 256])
                fl = dict(start=(i == 0), stop=(i == NCHUNK - 1))
                nc.tensor.matmul(t2T_ps[:], _r(p2f_sb[:, i * 128:(i + 1) * 128]),
                                 _r(t1_t[:]), **fl)
                for g in range(2):
                    nc.tensor.matmul(t3T_ps[g][:],
                                     _r(xt_t[:, g * 128:(g + 1) * 128]),
                                     _r(t1_t[:]), **fl)

            # ---- phase 1b: t4 chain (tiny) ----------------------------------
            t2T_sb = sb_small.tile([128, 256], F32, tag="t2Tsb")
            nc.vector.tensor_copy(t2T_sb[:], t2T_ps[:])
            for t in range(2):
                t2_ps = small_ps.tile([128, 128], F32, tag="smps")
                nc.tensor.transpose(t2_ps[:], t2T_sb[:, t * 128:(t + 1) * 128],
                                    id_sb[:])
                nc.vector.tensor_copy(t2p_sb[:, t * 134 + 3:t * 134 + 131],
                                      t2_ps[:])
            t4T_sb = sb_small.tile([128, 256], F32, tag="t4Tsb")
            for t in range(2):
                t4_ps = small_ps.tile([128, 128], F32, tag="smps")
                for ki, kh in enumerate(range(3)):
                    nc.tensor.matmul(
                        t4_ps[:], w4t_sb[:, (kh * 2 + t) * 128:(kh * 2 + t + 1) * 128],
                        t2p_sb[:, t * 134 + 3 * kh:t * 134 + 3 * kh + 128],
                        start=(ki == 0), stop=(ki == 2))
                t4_sb = sb_small.tile([128, 128], F32, tag="t4sb")
                nc.vector.tensor_copy(t4_sb[:], t4_ps[:])
                t4T_ps = small_ps.tile([128, 128], F32, tag="smps")
                nc.tensor.transpose(t4T_ps[:], t4_sb[:], id_sb[:])
                nc.vector.tensor_copy(t4T_sb[:, t * 128:(t + 1) * 128], t4T_ps[:])
            # t7T[g] = t4T - t3T[g]/56
            t7T_sb = [sb_small.tile([128, 256], F32R, name=f"t7T{g}", tag=f"t7T{g}")
                      for g in range(2)]
            for g in range(2):
                nc.vector.scalar_tensor_tensor(t7T_sb[g][:], t3T_ps[g][:], -inv56,
                                               t4T_sb[:], op0=MUL, op1=ADD)

            # ---- phase 2+3: t5/t8/t9 and final matmul, per column chunk -----
            xpad_t = xpad_pool.tile([128, 2 * HWPM], F32R)
            nc.sync.dma_start(
                xpad_t[:].rearrange("p (t f) -> p t f", t=2),
                d["xpad"][s].rearrange("t p f -> p t f"))
            for j in range(NCH2):
                c0, c1 = j * CH_SP, (j + 1) * CH_SP
                t9_sb = []
                for g in range(2):
                    # t8 first (double-buffered) so the next chunk's PE work
                    # never waits on the previous chunk's max-read of t5.
                    t8_ps = sp_ps.tile([128, CH_SP], F32, tag="t8", bufs=2)
                    for ti, (kh, kw) in enumerate(taps):
                        sh = 3 * PW * (kh - 1) + 3 * (kw - 1)
                        widx = (kh * 3 + kw) * 2 + g
                        ro = g * HWPM + XM + c0 + sh
                        nc.tensor.matmul(
                            t8_ps[:],
                            _r(w8t_sb[:, widx * 128:(widx + 1) * 128]),
                            _r(xpad_t[:, ro:ro + CH_SP]),
                            start=(ti == 0), stop=(ti == len(taps) - 1))
                    t5_ps = sp_ps.tile([128, CH_SP], F32, tag="t5", bufs=1)
                    for cc in range(2):
                        nc.tensor.matmul(
                            t5_ps[:],
                            _r(w5t_sb[:, (g * 2 + cc) * 128:(g * 2 + cc + 1) * 128]),
                            _r(xpad_t[:, cc * HWPM + XM + c0:cc * HWPM + XM + c1]),
                            start=(cc == 0), stop=(cc == 1))
                    # stage both PSUM results to SBUF with plain 2D copies
                    # (walrus rejects TensorTensor with two PSUM operands),
                    # then max + compact 62 -> 56 cols per row in SBUF.
                    t5_sb = out_pool.tile([128, CH_SP], F32, tag="t5sb", bufs=2)
                    nc.scalar.copy(t5_sb[:], t5_ps[:])
                    t8_sb = out_pool.tile([128, CH_SP], F32, tag="t8sb", bufs=2)
                    nc.vector.tensor_copy(t8_sb[:], t8_ps[:])
                    t9_g = t9_pool.tile([128, CH_OUT], F32R, name=f"t9g{g}",
                                        tag="t9c")
                    nc.vector.tensor_max(
                        t9_g[:].rearrange("p (r c) -> p r c", c=56),
                        t5_sb[:].rearrange("p (r c) -> p r c", c=62)[:, :, 3:59],
                        t8_sb[:].rearrange("p (r c) -> p r c", c=62)[:, :, 3:59])
                    t9_sb.append(t9_g)
                for ct in range(2):
                    o_ps = small_ps.tile([128, CH_OUT], F32, name="o_ps",
                                         tag="smps")
                    for g in range(2):
                        nc.tensor.matmul(
                            o_ps[:],
                            _r(t7T_sb[g][:, ct * 128:(ct + 1) * 128]),
                            _r(t9_sb[g][:]),
                            start=(g == 0), stop=(g == 1))
                    o_sb = out_pool.tile([128, CH_OUT], F32, tag="osb")
                    nc.scalar.mul(o_sb[:], o_ps[:], inv16)
                    nc.sync.dma_start(
                        out_dram[s, ct, :, j * CH_OUT:(j + 1) * CH_OUT], o_sb[:])


# ---------------------------------------------------------------------------
# host-side input preparation
# ---------------------------------------------------------------------------

def _prep_shared(p1, p2, w4, w5, w8):
    p1 = np.asarray(p1, np.float32)[0]          # [C,H,W]
    p2 = np.asarray(p2, np.float32)[..., 0]     # [H,W,K]
    w4 = np.asarray(w4, np.float32)
    w5 = np.asarray(w5, np.float32)
    w8 = np.asarray(w8, np.float32)

    p1t = np.zeros((HWPAD, 256), np.float32)
    p1t[:HW] = p1.reshape(C, HW).T
    p2f = np.zeros((HWPAD, 128), np.float32)
    p2f[:HW] = p2.reshape(HW, 128)

    def blockdiag_T(w, kh, kw):
        # out[t][ci, co] = w[t*128+co, ci_local, kh, kw] iff ci//4 == co//4
        out = np.zeros((2, 32, 4, 32, 4), np.float32)
        v = w.reshape(2, 32, 4, 4, 3, 3)        # [t, grp, co_l, ci_l, kh, kw]
        r = np.arange(32)
        out[:, r, :, r, :] = v[:, :, :, :, kh, kw].transpose(1, 0, 3, 2)
        return out.reshape(2, 128, 128)

    w4t = np.stack([blockdiag_T(w4, kh, 1) for kh in range(3)])          # [3,2,...]
    w8t = np.stack([np.stack([blockdiag_T(w8, kh, kw) for kw in range(3)])
                    for kh in range(3)])                                  # [3,3,2,...]
    w5t = np.zeros((2, 2, 128, 128), np.float32)
    for dt_ in range(2):
        for cc in range(2):
            w5t[dt_, cc] = w5[dt_ * 128:(dt_ + 1) * 128,
                              cc * 128:(cc + 1) * 128].T
    ident = np.eye(128, dtype=np.float32)
    return dict(p1t=p1t.reshape(NCHUNK, 128, 256),
                p2f=p2f.reshape(NCHUNK, 128, 128),
                w4t=w4t, w5t=w5t, w8t=w8t, ident=ident)


def _prep_core(x_shard):
    # x_shard: [SPC, C, H, W]
    xs = np.asarray(x_shard, np.float32)
    xpad = np.zeros((SPC, 2, 128, HWPM), np.float32)
    xpw = np.zeros((SPC, 2, 128, H, PW), np.float32)
    xpw[:, :, :, :, 3:3 + W] = xs.reshape(SPC, 2, 128, H, W)
    xpad[:, :, :, XM:XM + HWP] = xpw.reshape(SPC, 2, 128, HWP)
    xt = np.zeros((SPC, HWPAD, 256), np.float32)
    xt[:, :HW] = xs.reshape(SPC, C, HW).transpose(0, 2, 1)
    return dict(xpad=xpad,
                xt=xt.reshape(SPC, NCHUNK, 128, 256))


def kernel(x, p1, p2, w4, w5, w8):
    if "nc" not in _PROGRAM_CACHE:
        _PROGRAM_CACHE["nc"] = _build_program()
    nc = _PROGRAM_CACHE["nc"]

    shared = _prep_shared(p1, p2, w4, w5, w8)
    x = np.asarray(x, np.float32)
    in_maps = []
    for c in range(NCORE):
        m = dict(shared)
        m.update(_prep_core(x[c * SPC:(c + 1) * SPC]))
        in_maps.append(m)

    res = run_bass_kernel_spmd(nc, in_maps, core_ids=list(range(NCORE)))
    outs = []
    for c in range(NCORE):
        o = res.results[c]["out"]               # [SPC, 2, 128, HW]
        outs.append(o.reshape(SPC, C, H, W))
    return np.concatenate(outs, axis=0)

